# revision 17
# baseline (speedup 1.0000x reference)
"""Mixture-of-Experts (B=4, S=2048, D=1024, F=4096, E=8, top-2) on 8 trn2 NeuronCores.

Strategy: grouped expert F-slicing for near-perfect load balance at low DMA
traffic.
  Cores are split into 2 groups of 4. Each group serves 4 experts; every core
  in a group processes ALL tokens dispatched to those experts, but only a
  1024-wide slice of the expert hidden dim F (gelu is elementwise, so the
  F-split is exact): partial y = gelu(x @ W1[:, sl] + b1[sl]) @ W2[sl, :].
  The host sums the 4 partial y's per group, adds b2, applies the gate
  combine weights and scatter-adds.

  Load balance: experts are ranked by dispatch count; group 1 takes ranks
  {1,3,5,7}, group 2 ranks {2,4,6,8}, and slot j's compile-time capacity is
  the rank-(2j-1) count — the provably minimal per-core token total under
  one shared SPMD program (sum of odd-rank counts, /4 per core).

  DMA: every transfer is contiguous on BOTH the DRAM and SBUF side (the host
  pre-swizzles x, w1, w2 into per-partition-contiguous blocks; y returns in
  a swizzled layout the host undoes). This gives ~8-16KB DMA runs per
  partition instead of 1KB, which keeps the per-core descriptor rate well
  under the fabric limit that throttled an earlier 8-way-split version.
  Inputs issue on the Sync engine queue, outputs on the GpSimd engine queue
  so outputs never convoy behind input triggers.

  PE: chunks (<=512 tokens) are software-pipelined — mm1 of chunk i+1 is
  emitted before mm2 of chunk i so the PE never waits on a chunk's last
  gelu; ~36 dummy warm-up matmuls on zeroed SBUF run during the initial DMA
  fill so the HAM clock is at 2.4 GHz when real work starts; weight tiles
  are 2D so LDWEIGHTS keeps Fast-Weight-Load (a 3D stationary AP measurably
  disables it: +30ns per matmul).
"""

import copy
import sys

import numpy as np

for _p in ("/opt/trn_rl_repo", "/opt/pypackages"):
    if _p not in sys.path:
        sys.path.append(_p)

import ml_dtypes

B, S, D = 4, 2048, 1024
F = 4 * D
E = 8
TOP_K = 2
P = 128
G = 1                # core groups
EPG = E // G         # experts per group (= slots)
FS = F // 8          # 512: per-core F slice (8 cores, one group)
C_CHUNK = 512
N_DUMMY = 10

KO = D // P          # 8 k-subtiles for mm1
FT = FS // P         # 8 f-tiles per expert slice
DT = D // P          # 8 d-tiles of the partial y

# test-harness hooks (left off for grading)
TRACE = False
LAST_RESULTS = None

_compiled = {}


def _split_drain_waits(nc, max_waits=1):
    """This walrus build rejects instructions carrying more than one sync
    wait ("Too many sync wait commands"). Keep one wait on the instruction and
    move the excess onto NoOps inserted right before it on the same engine
    (engines are in-order, so blocking semantics are identical). Updates stay
    on the original instruction — moving them to a trailing NoOp could signal
    before the op's writes land."""
    import concourse.mybir as mybir

    m = nc.m
    new_module = copy.replace(m, functions=[])
    for function in m.functions:
        new_function = copy.replace(function, blocks=[])
        new_function.set_allocations_from_list(function.allocations)
        for block in function.blocks:
            out = []
            for inst in block.instructions:
                si = getattr(inst, "sync_info", None)
                on_wait = list(si.on_wait) if si is not None and si.on_wait else []
                if len(on_wait) > max_waits:
                    engine = getattr(inst, "engine", None)
                    extra, keep = on_wait[max_waits:], on_wait[:max_waits]
                    for j, w in enumerate(extra):
                        out.append(
                            mybir.InstNoOp(
                                name=f"{inst.name}-w{j}",
                                engine=engine,
                                sync_info=mybir.SyncInfo(on_wait=[w], on_update=[]),
                                bass_nofuse=True,
                            )
                        )
                    inst.sync_info = mybir.SyncInfo(
                        on_wait=keep,
                        on_update=list(si.on_update) if si.on_update else [],
                    )
                out.append(inst)
            new_function.blocks.append(copy.replace(block, instructions=out))
        new_module.functions.append(new_function)
    nc.m = new_module
    return nc


def _chunk_plan(caps):
    """Flat list of (slot, t0, cn) chunks. Per-slot chunk sizes balanced
    (<=512, diff<=1); the final chunk overall is carved to 128 tokens to
    shrink the trailing DMA after the last matmul."""
    plan = []
    t0 = 0
    for j, cap in enumerate(caps):
        cap = int(cap)
        if cap == 0:
            continue
        tails = []
        if j == len(caps) - 1 and cap > 640:
            tails = [128, 128]          # taper the final output drain
            cap -= 256
        n = -(-cap // C_CHUNK)
        base, rem = divmod(cap, n)
        for i in range(n):
            cn = base + (1 if i < rem else 0)
            plan.append((j, t0, cn))
            t0 += cn
        for tail in tails:
            plan.append((j, t0, tail))
            t0 += tail
    return plan


def _build_nc(caps):
    import concourse.bass as bass
    import concourse.mybir as mybir
    from concourse.tile import TileContext

    fp32 = mybir.dt.float32
    bf16 = mybir.dt.bfloat16
    AF = mybir.ActivationFunctionType

    T = int(sum(caps))
    plan = _chunk_plan(caps)

    nc = bass.Bass()
    # All DRAM layouts are host-swizzled so every DMA is contiguous per
    # partition on both sides:
    #   x_d[p, KO*t0 + ko*cn + c]   = x[ko*128+p, t0+c]   (per chunk)
    #   w1_d[p, (j*KO+ko)*FS + f]   = W1slice[j][ko*128+p, f]
    #   w2_d[p, (j*FT+lf)*D + d]    = W2slice[j][lf*128+p, d]
    #   y_d[p, DT*t0 + dt*cn + c]   = y[dt*128+p, t0+c]   (per chunk)
    x_d = nc.declare_dram_parameter("x", [P, KO * T], bf16, isOutput=False)
    w1_d = nc.declare_dram_parameter("w1", [P, EPG * KO * FS], bf16, isOutput=False)
    w2_d = nc.declare_dram_parameter("w2", [P, EPG * FT * D], bf16, isOutput=False)
    b1_d = nc.declare_dram_parameter("b1", [P, EPG * FT], fp32, isOutput=False)
    y_d = nc.declare_dram_parameter("y", [P, DT * T], bf16, isOutput=True)

    with TileContext(nc) as tc:
        with (
            tc.tile_pool(name="wpool", bufs=1) as wpool,
            tc.tile_pool(name="xpool", bufs=3) as xpool,
            tc.tile_pool(name="hpool", bufs=2) as hpool,
            tc.tile_pool(name="ypool", bufs=2) as ypool,
            tc.tile_pool(name="hpsum", bufs=4, space="PSUM") as hpsum,
            tc.tile_pool(name="ypsum", bufs=4, space="PSUM") as ypsum,
        ):
            # --- PE warm-up on zeroed SBUF; result never read. Scratch PSUM
            # is the first "yps" buffer; real groups reuse it behind them.
            dum_w = wpool.tile([P, C_CHUNK], bf16)
            nc.vector.memset(dum_w[:], 0)
            dum_ps = ypsum.tile([P, C_CHUNK], fp32, tag="yps")
            for _ in range(N_DUMMY):
                nc.tensor.matmul(
                    dum_ps[:], dum_w[:, :P], dum_w[:], start=True, stop=True
                )

            w1_t = [None] * EPG    # [P, KO*FS] 2D per slot
            w2_t = [None] * EPG    # [P, FT*D] 2D per slot
            x_sb = [None] * len(plan)

            def load_x(ci, split=1):
                j, t0, cn = plan[ci]
                t = xpool.tile([P, KO * C_CHUNK], bf16, tag="x")
                step = -(-KO // split)
                for k0 in range(0, KO, step):
                    k1 = min(k0 + step, KO)
                    nc.sync.dma_start(
                        t[:, k0 * cn:k1 * cn],
                        x_d[:, KO * t0 + k0 * cn:KO * t0 + k1 * cn])
                x_sb[ci] = t

            def load_w1(j, kos):
                # per-ko pieces: 2KB contiguous runs per partition, and the
                # first matmuls can begin as soon as their piece lands
                if w1_t[j] is None:
                    t1 = wpool.tile([P, KO * FS], bf16, tag=f"w1_{j}")
                    w1_t[j] = t1
                t1 = w1_t[j]
                for ko in kos:
                    nc.sync.dma_start(
                        t1[:, ko * FS:(ko + 1) * FS],
                        w1_d[:, (j * KO + ko) * FS:(j * KO + ko + 1) * FS])

            def load_w2(j):
                t2 = wpool.tile([P, FT * D], bf16, tag=f"w2_{j}")
                for lf in range(FT):
                    nc.sync.dma_start(
                        t2[:, lf * D:(lf + 1) * D],
                        w2_d[:, (j * FT + lf) * D:(j * FT + lf + 1) * D])
                w2_t[j] = t2

            # startup: just enough bytes ahead of each consumer — w1[slot0]
            # ko0 piece and the first half of x0 gate the first real matmul.
            X_AHEAD = 2            # <= xpool bufs - 1
            j0 = plan[0][0]
            load_w1(j0, [0])
            load_x(0, split=4)
            load_w1(j0, range(1, KO))
            b1_sb = wpool.tile([P, EPG * FT], fp32)
            nc.sync.dma_start(b1_sb[:], b1_d[:])
            if len(plan) > 1:
                load_x(1)
            load_w2(j0)
            if len(plan) > 2:
                load_x(2)
            next_w = j0 + 1

            def load_w_through(j):
                nonlocal next_w
                while next_w <= min(j, EPG - 1):
                    load_w1(next_w, range(KO))
                    load_w2(next_w)
                    next_w += 1

            load_w_through(j0 + 1)

            h_of = [None] * len(plan)

            def mm1(ci):
                j, t0, cn = plan[ci]
                h_sb = hpool.tile([P, FT, C_CHUNK], bf16, tag="h")
                for ft in range(FT):
                    h_ps = hpsum.tile([P, C_CHUNK], fp32, tag="hps")
                    for ko in range(KO):
                        nc.tensor.matmul(
                            h_ps[:, :cn],
                            w1_t[j][:, ko * FS + ft * P:ko * FS + (ft + 1) * P],
                            x_sb[ci][:, ko * cn:(ko + 1) * cn],
                            start=(ko == 0),
                            stop=(ko == KO - 1),
                        )
                    nc.scalar.activation(
                        h_sb[:, ft, :cn], h_ps[:, :cn], AF.Gelu,
                        bias=b1_sb[:, j * FT + ft:j * FT + ft + 1],
                    )
                h_of[ci] = h_sb

            def mm2(ci):
                j, t0, cn = plan[ci]
                h_sb = h_of[ci]
                y_sb = ypool.tile([P, DT * C_CHUNK], bf16, tag="y")
                for dt_ in range(DT):
                    y_ps = ypsum.tile([P, C_CHUNK], fp32, tag="yps")
                    for lf in range(FT):
                        nc.tensor.matmul(
                            y_ps[:, :cn],
                            w2_t[j][:, lf * D + dt_ * P:lf * D + (dt_ + 1) * P],
                            h_sb[:, lf, :cn],
                            start=(lf == 0),
                            stop=(lf == FT - 1),
                        )
                    nc.vector.tensor_copy(
                        y_sb[:, dt_ * cn:(dt_ + 1) * cn], y_ps[:, :cn])
                nc.gpsimd.dma_start(
                    y_d[:, DT * t0:DT * (t0 + cn)], y_sb[:, :DT * cn])
                h_of[ci] = None

            mm1(0)
            for ci in range(1, len(plan)):
                if ci + X_AHEAD < len(plan):
                    load_x(ci + X_AHEAD)
                if ci + 1 < len(plan) and plan[ci + 1][0] != plan[ci][0]:
                    load_w_through(plan[ci + 1][0] + 1)
                mm1(ci)
                mm2(ci - 1)
            mm2(len(plan) - 1)

    return _split_drain_waits(nc)


def _to_bf16(a):
    """Fast float32 -> bfloat16 with round-to-nearest-even via bit ops."""
    a = np.ascontiguousarray(a, dtype=np.float32)
    u = a.view(np.uint32)
    r = ((u + 0x7FFF + ((u >> 16) & 1)) >> 16).astype(np.uint16)
    return r.view(ml_dtypes.bfloat16)


def _swizzle_rows(m):
    """[R*128, C] -> [128, R*C] with block r of C cols holding rows
    r*128..r*128+127 (per-partition contiguous layout)."""
    R = m.shape[0] // P
    return np.ascontiguousarray(
        m.reshape(R, P, m.shape[1]).transpose(1, 0, 2).reshape(P, -1))


def kernel(hidden_states, Wg, bg, W1, b1, W2, b2):
    from concourse import bass_utils

    hs = np.ascontiguousarray(hidden_states, dtype=np.float32).reshape(B * S, D)

    # ---- Gate on host (float64): softmax over experts, top-2, renormalize
    logits = hs.astype(np.float64) @ np.asarray(Wg, np.float64).T
    logits += np.asarray(bg, np.float64)
    logits -= logits.max(axis=-1, keepdims=True)
    p = np.exp(logits)
    p /= p.sum(axis=-1, keepdims=True)

    i1 = p.argmax(axis=-1)
    rows = np.arange(B * S)
    p1 = p[rows, i1]
    pm = p.copy()
    pm[rows, i1] = -1.0
    i2 = pm.argmax(axis=-1)
    p2 = p[rows, i2]
    denom = p1 + p2
    g1 = (p1 / denom).astype(np.float32)
    g2 = (p2 / denom).astype(np.float32)

    ids, cws = [], []
    for e in range(E):
        m1 = np.nonzero(i1 == e)[0]
        m2 = np.nonzero(i2 == e)[0]
        ids.append(np.concatenate([m1, m2]))
        cws.append(np.concatenate([g1[m1], g2[m2]]))
    counts = np.array([len(x) for x in ids])

    # ---- Single group: all 8 cores share all experts (F split 8 ways);
    # slot capacities are the exact per-expert counts — perfect balance.
    rank = np.argsort(-counts, kind="stable")
    groups = [[int(rank[j]) for j in range(EPG)]]
    caps = tuple(int(counts[rank[j]]) for j in range(EPG))
    T = int(sum(caps))
    plan = _chunk_plan(caps)

    if caps not in _compiled:
        _compiled[caps] = _build_nc(caps)
    nc = _compiled[caps]

    hs_b = _to_bf16(hs)

    # ---- Per-group token matrices (swizzled per-chunk, shared by 4 cores)
    x_g, off_g = [], []
    for g in range(G):
        xg = np.zeros((D, T), dtype=ml_dtypes.bfloat16)
        offs = []
        t0 = 0
        for j in range(EPG):
            e = groups[g][j]
            cnt = counts[e]
            xg[:, t0:t0 + cnt] = hs_b[ids[e]].T
            offs.append(t0)
            t0 += caps[j]
        off_g.append(offs)
        # swizzle: per chunk ci at t0 with cn cols: [D, cn] -> [128, KO*cn]
        xd = np.empty((P, KO * T), dtype=ml_dtypes.bfloat16)
        for (j, t0, cn) in plan:
            blk = xg[:, t0:t0 + cn].reshape(KO, P, cn)
            xd[:, KO * t0:KO * (t0 + cn)] = (
                blk.transpose(1, 0, 2).reshape(P, KO * cn))
        x_g.append(np.ascontiguousarray(xd))

    w1_all = _to_bf16(W1)                       # [E, D, F]
    w2_all = _to_bf16(W2)                       # [E, F, D]
    b1_all = np.asarray(b1, np.float32)         # [E, F]

    in_maps = []
    for c in range(E):
        g, k = 0, c                             # group, F-slice index
        sl = slice(k * FS, (k + 1) * FS)
        w1_c = np.concatenate(
            [_swizzle_rows(np.ascontiguousarray(
                w1_all[e, :, sl])) for e in groups[g]], axis=1)
        w2_c = np.concatenate(
            [_swizzle_rows(np.ascontiguousarray(
                w2_all[e, sl, :])) for e in groups[g]], axis=1)
        b1_c = np.concatenate(
            [np.ascontiguousarray(
                b1_all[e, sl].reshape(FT, P).T) for e in groups[g]], axis=1)
        in_maps.append({
            "x": x_g[g],
            "w1": np.ascontiguousarray(w1_c),
            "w2": np.ascontiguousarray(w2_c),
            "b1": np.ascontiguousarray(b1_c),
        })

    kwargs = {}
    if TRACE:
        import os as _os
        kwargs = dict(trace=True, trace_cores=list(range(E)))
        if _os.environ.get("MOE_TRACE_DIR"):
            _os.makedirs(_os.environ["MOE_TRACE_DIR"], exist_ok=True)
            kwargs["tmpdir"] = _os.environ["MOE_TRACE_DIR"]
    res = bass_utils.run_bass_kernel_spmd(nc, in_maps, list(range(E)), **kwargs)
    global LAST_RESULTS
    LAST_RESULTS = res

    # ---- Combine: per group sum the 4 partial y's, un-swizzle, add b2,
    # apply combine weights, scatter-add
    out = np.zeros((B * S, D), dtype=np.float32)
    b2_f = np.asarray(b2, np.float32)
    for g in range(G):
        acc = np.zeros((P, DT * T), dtype=np.float32)
        for k in range(8):
            acc += res.results[g * 8 + k]["y"].astype(np.float32)
        yg = np.empty((D, T), dtype=np.float32)
        for (j, t0, cn) in plan:
            blk = acc[:, DT * t0:DT * (t0 + cn)].reshape(P, DT, cn)
            yg[:, t0:t0 + cn] = blk.transpose(1, 0, 2).reshape(D, cn)
        for j in range(EPG):
            e = groups[g][j]
            cnt = counts[e]
            y_e = yg[:, off_g[g][j]:off_g[g][j] + cnt].T + b2_f[e]
            out[ids[e]] += cws[e][:, None] * y_e
    return out.reshape(B, S, D)


# revision 18
# speedup vs baseline: 1.0068x; 1.0068x over previous
"""Mixture-of-Experts (B=4, S=2048, D=1024, F=4096, E=8, top-2) on 8 trn2 NeuronCores.

Strategy: full expert F-slicing for perfect load balance.
  Every core processes ALL 16384 dispatched (token, expert) pairs, but only
  a 512-wide slice of the expert hidden dim F (gelu is elementwise, so the
  F-split is exact): partial y = gelu(x @ W1[:, sl] + b1[sl]) @ W2[sl, :].
  The host sums the 8 partial y's, adds b2, applies the gate combine
  weights and scatter-adds. Per-core work is exactly 2048 token-equivalents
  (vs. 2176 = padded max expert count for one-expert-per-core), which puts
  the matmul roofline at 16384*128cyc/2.4GHz = 437us.

  DMA: every transfer is contiguous on BOTH the DRAM and SBUF side (the host
  pre-swizzles x, w1, w2 into per-partition-contiguous blocks; y returns in
  a swizzled layout the host undoes). This gives ~7-16KB DMA runs per
  partition instead of 1KB, which keeps the per-core descriptor rate under
  the fabric limit that throttled a naive row-major version of this split
  (~84MB/core of traffic). Inputs issue on the Sync engine queue, outputs on
  the GpSimd engine queue so outputs never convoy behind input triggers
  whose buffer-free semaphores gate on compute progress.

  PE: chunks (<=512 tokens) are software-pipelined — mm1 of chunk i+1 is
  emitted before mm2 of chunk i so the PE never waits on a chunk's last
  gelu; a few dummy warm-up matmuls on zeroed SBUF run during the initial
  DMA fill so the HAM clock gate is releasing by the time real work starts;
  weight tiles are 2D so LDWEIGHTS keeps Fast-Weight-Load (a 3D stationary
  AP measurably disables it: +30ns per matmul); the final two chunks are
  128 tokens to taper the trailing output drain.

  Measured: 470us (baseline expert-parallel kernel: 495us).
"""

import copy
import sys

import numpy as np

for _p in ("/opt/trn_rl_repo", "/opt/pypackages"):
    if _p not in sys.path:
        sys.path.append(_p)

import ml_dtypes

B, S, D = 4, 2048, 1024
F = 4 * D
E = 8
TOP_K = 2
P = 128
G = 1                # core groups
EPG = E // G         # experts per group (= slots)
FS = F // 8          # 512: per-core F slice (8 cores, one group)
C_CHUNK = 512
N_DUMMY = 10

KO = D // P          # 8 k-subtiles for mm1
FT = FS // P         # 8 f-tiles per expert slice
DT = D // P          # 8 d-tiles of the partial y

# test-harness hooks (left off for grading)
TRACE = False
LAST_RESULTS = None

_compiled = {}


def _split_drain_waits(nc, max_waits=1):
    """This walrus build rejects instructions carrying more than one sync
    wait ("Too many sync wait commands"). Keep one wait on the instruction and
    move the excess onto NoOps inserted right before it on the same engine
    (engines are in-order, so blocking semantics are identical). Updates stay
    on the original instruction — moving them to a trailing NoOp could signal
    before the op's writes land."""
    import concourse.mybir as mybir

    m = nc.m
    new_module = copy.replace(m, functions=[])
    for function in m.functions:
        new_function = copy.replace(function, blocks=[])
        new_function.set_allocations_from_list(function.allocations)
        for block in function.blocks:
            out = []
            for inst in block.instructions:
                si = getattr(inst, "sync_info", None)
                on_wait = list(si.on_wait) if si is not None and si.on_wait else []
                if len(on_wait) > max_waits:
                    engine = getattr(inst, "engine", None)
                    extra, keep = on_wait[max_waits:], on_wait[:max_waits]
                    for j, w in enumerate(extra):
                        out.append(
                            mybir.InstNoOp(
                                name=f"{inst.name}-w{j}",
                                engine=engine,
                                sync_info=mybir.SyncInfo(on_wait=[w], on_update=[]),
                                bass_nofuse=True,
                            )
                        )
                    inst.sync_info = mybir.SyncInfo(
                        on_wait=keep,
                        on_update=list(si.on_update) if si.on_update else [],
                    )
                out.append(inst)
            new_function.blocks.append(copy.replace(block, instructions=out))
        new_module.functions.append(new_function)
    nc.m = new_module
    return nc


def _chunk_plan(caps):
    """Flat list of (slot, t0, cn) chunks. Per-slot chunk sizes balanced
    (<=512, diff<=1); the final chunk overall is carved to 128 tokens to
    shrink the trailing DMA after the last matmul."""
    plan = []
    t0 = 0
    for j, cap in enumerate(caps):
        cap = int(cap)
        if cap == 0:
            continue
        tails = []
        if j == len(caps) - 1 and cap > 640:
            tails = [128, 128]          # taper the final output drain
            cap -= 256
        n = -(-cap // C_CHUNK)
        base, rem = divmod(cap, n)
        for i in range(n):
            cn = base + (1 if i < rem else 0)
            plan.append((j, t0, cn))
            t0 += cn
        for tail in tails:
            plan.append((j, t0, tail))
            t0 += tail
    return plan


def _build_nc(caps):
    import concourse.bass as bass
    import concourse.mybir as mybir
    from concourse.tile import TileContext

    fp32 = mybir.dt.float32
    bf16 = mybir.dt.bfloat16
    AF = mybir.ActivationFunctionType

    T = int(sum(caps))
    plan = _chunk_plan(caps)

    nc = bass.Bass()
    # All DRAM layouts are host-swizzled so every DMA is contiguous per
    # partition on both sides:
    #   x_d[p, KO*t0 + ko*cn + c]   = x[ko*128+p, t0+c]   (per chunk)
    #   w1_d[p, (j*KO+ko)*FS + f]   = W1slice[j][ko*128+p, f]
    #   w2_d[p, (j*FT+lf)*D + d]    = W2slice[j][lf*128+p, d]
    #   y_d[p, DT*t0 + dt*cn + c]   = y[dt*128+p, t0+c]   (per chunk)
    x_d = nc.declare_dram_parameter("x", [P, KO * T], bf16, isOutput=False)
    w1_d = nc.declare_dram_parameter("w1", [P, EPG * KO * FS], bf16, isOutput=False)
    w2_d = nc.declare_dram_parameter("w2", [P, EPG * FT * D], bf16, isOutput=False)
    b1_d = nc.declare_dram_parameter("b1", [P, EPG * FT], fp32, isOutput=False)
    y_d = nc.declare_dram_parameter("y", [P, DT * T], bf16, isOutput=True)

    with TileContext(nc) as tc:
        with (
            tc.tile_pool(name="wpool", bufs=1) as wpool,
            tc.tile_pool(name="xpool", bufs=3) as xpool,
            tc.tile_pool(name="hpool", bufs=2) as hpool,
            tc.tile_pool(name="ypool", bufs=2) as ypool,
            tc.tile_pool(name="hpsum", bufs=4, space="PSUM") as hpsum,
            tc.tile_pool(name="ypsum", bufs=4, space="PSUM") as ypsum,
        ):
            # --- PE warm-up on zeroed SBUF; result never read. Scratch PSUM
            # is the first "yps" buffer; real groups reuse it behind them.
            dum_w = wpool.tile([P, C_CHUNK], bf16)
            nc.vector.memset(dum_w[:], 0)
            dum_ps = ypsum.tile([P, C_CHUNK], fp32, tag="yps")
            for _ in range(N_DUMMY):
                nc.tensor.matmul(
                    dum_ps[:], dum_w[:, :P], dum_w[:], start=True, stop=True
                )

            w1_t = [None] * EPG    # [P, KO*FS] 2D per slot
            w2_t = [None] * EPG    # [P, FT*D] 2D per slot
            x_sb = [None] * len(plan)

            def load_x(ci, split=1):
                j, t0, cn = plan[ci]
                t = xpool.tile([P, KO * C_CHUNK], bf16, tag="x")
                step = -(-KO // split)
                for k0 in range(0, KO, step):
                    k1 = min(k0 + step, KO)
                    nc.sync.dma_start(
                        t[:, k0 * cn:k1 * cn],
                        x_d[:, KO * t0 + k0 * cn:KO * t0 + k1 * cn])
                x_sb[ci] = t

            def load_w1(j, kos):
                # per-ko pieces: 2KB contiguous runs per partition, and the
                # first matmuls can begin as soon as their piece lands
                if w1_t[j] is None:
                    t1 = wpool.tile([P, KO * FS], bf16, tag=f"w1_{j}")
                    w1_t[j] = t1
                t1 = w1_t[j]
                for ko in kos:
                    nc.sync.dma_start(
                        t1[:, ko * FS:(ko + 1) * FS],
                        w1_d[:, (j * KO + ko) * FS:(j * KO + ko + 1) * FS])

            def load_w2(j):
                t2 = wpool.tile([P, FT * D], bf16, tag=f"w2_{j}")
                for lf in range(FT):
                    nc.sync.dma_start(
                        t2[:, lf * D:(lf + 1) * D],
                        w2_d[:, (j * FT + lf) * D:(j * FT + lf + 1) * D])
                w2_t[j] = t2

            # startup: just enough bytes ahead of each consumer — w1[slot0]
            # ko0 piece and the first half of x0 gate the first real matmul.
            X_AHEAD = 2            # <= xpool bufs - 1
            j0 = plan[0][0]
            load_w1(j0, [0])
            load_x(0, split=4)
            load_w1(j0, range(1, KO))
            b1_sb = wpool.tile([P, EPG * FT], fp32)
            nc.sync.dma_start(b1_sb[:], b1_d[:])
            if len(plan) > 1:
                load_x(1)
            load_w2(j0)
            if len(plan) > 2:
                load_x(2)
            next_w = j0 + 1

            def load_w_through(j):
                nonlocal next_w
                while next_w <= min(j, EPG - 1):
                    load_w1(next_w, range(KO))
                    load_w2(next_w)
                    next_w += 1

            load_w_through(j0 + 1)

            h_of = [None] * len(plan)

            def mm1(ci):
                j, t0, cn = plan[ci]
                h_sb = hpool.tile([P, FT, C_CHUNK], bf16, tag="h")
                for ft in range(FT):
                    h_ps = hpsum.tile([P, C_CHUNK], fp32, tag="hps")
                    for ko in range(KO):
                        nc.tensor.matmul(
                            h_ps[:, :cn],
                            w1_t[j][:, ko * FS + ft * P:ko * FS + (ft + 1) * P],
                            x_sb[ci][:, ko * cn:(ko + 1) * cn],
                            start=(ko == 0),
                            stop=(ko == KO - 1),
                        )
                    nc.scalar.activation(
                        h_sb[:, ft, :cn], h_ps[:, :cn], AF.Gelu,
                        bias=b1_sb[:, j * FT + ft:j * FT + ft + 1],
                    )
                h_of[ci] = h_sb

            def mm2(ci):
                j, t0, cn = plan[ci]
                h_sb = h_of[ci]
                y_sb = ypool.tile([P, DT * C_CHUNK], bf16, tag="y")
                for dt_ in range(DT):
                    y_ps = ypsum.tile([P, C_CHUNK], fp32, tag="yps")
                    for lf in range(FT):
                        nc.tensor.matmul(
                            y_ps[:, :cn],
                            w2_t[j][:, lf * D + dt_ * P:lf * D + (dt_ + 1) * P],
                            h_sb[:, lf, :cn],
                            start=(lf == 0),
                            stop=(lf == FT - 1),
                        )
                    nc.vector.tensor_copy(
                        y_sb[:, dt_ * cn:(dt_ + 1) * cn], y_ps[:, :cn])
                nc.gpsimd.dma_start(
                    y_d[:, DT * t0:DT * (t0 + cn)], y_sb[:, :DT * cn])
                h_of[ci] = None

            mm1(0)
            for ci in range(1, len(plan)):
                if ci + X_AHEAD < len(plan):
                    load_x(ci + X_AHEAD)
                if ci + 1 < len(plan) and plan[ci + 1][0] != plan[ci][0]:
                    load_w_through(plan[ci + 1][0] + 1)
                mm1(ci)
                mm2(ci - 1)
            mm2(len(plan) - 1)

    return _split_drain_waits(nc)


def _to_bf16(a):
    """Fast float32 -> bfloat16 with round-to-nearest-even via bit ops."""
    a = np.ascontiguousarray(a, dtype=np.float32)
    u = a.view(np.uint32)
    r = ((u + 0x7FFF + ((u >> 16) & 1)) >> 16).astype(np.uint16)
    return r.view(ml_dtypes.bfloat16)


def _swizzle_rows(m):
    """[R*128, C] -> [128, R*C] with block r of C cols holding rows
    r*128..r*128+127 (per-partition contiguous layout)."""
    R = m.shape[0] // P
    return np.ascontiguousarray(
        m.reshape(R, P, m.shape[1]).transpose(1, 0, 2).reshape(P, -1))


def kernel(hidden_states, Wg, bg, W1, b1, W2, b2):
    from concourse import bass_utils

    hs = np.ascontiguousarray(hidden_states, dtype=np.float32).reshape(B * S, D)

    # ---- Gate on host (float64): softmax over experts, top-2, renormalize
    logits = hs.astype(np.float64) @ np.asarray(Wg, np.float64).T
    logits += np.asarray(bg, np.float64)
    logits -= logits.max(axis=-1, keepdims=True)
    p = np.exp(logits)
    p /= p.sum(axis=-1, keepdims=True)

    i1 = p.argmax(axis=-1)
    rows = np.arange(B * S)
    p1 = p[rows, i1]
    pm = p.copy()
    pm[rows, i1] = -1.0
    i2 = pm.argmax(axis=-1)
    p2 = p[rows, i2]
    denom = p1 + p2
    g1 = (p1 / denom).astype(np.float32)
    g2 = (p2 / denom).astype(np.float32)

    ids, cws = [], []
    for e in range(E):
        m1 = np.nonzero(i1 == e)[0]
        m2 = np.nonzero(i2 == e)[0]
        ids.append(np.concatenate([m1, m2]))
        cws.append(np.concatenate([g1[m1], g2[m2]]))
    counts = np.array([len(x) for x in ids])

    # ---- Single group: all 8 cores share all experts (F split 8 ways);
    # slot capacities are the exact per-expert counts — perfect balance.
    rank = np.argsort(-counts, kind="stable")
    groups = [[int(rank[j]) for j in range(EPG)]]
    caps = tuple(int(counts[rank[j]]) for j in range(EPG))
    T = int(sum(caps))
    plan = _chunk_plan(caps)

    if caps not in _compiled:
        _compiled[caps] = _build_nc(caps)
    nc = _compiled[caps]

    hs_b = _to_bf16(hs)

    # ---- Per-group token matrices (swizzled per-chunk, shared by 4 cores)
    x_g, off_g = [], []
    for g in range(G):
        xg = np.zeros((D, T), dtype=ml_dtypes.bfloat16)
        offs = []
        t0 = 0
        for j in range(EPG):
            e = groups[g][j]
            cnt = counts[e]
            xg[:, t0:t0 + cnt] = hs_b[ids[e]].T
            offs.append(t0)
            t0 += caps[j]
        off_g.append(offs)
        # swizzle: per chunk ci at t0 with cn cols: [D, cn] -> [128, KO*cn]
        xd = np.empty((P, KO * T), dtype=ml_dtypes.bfloat16)
        for (j, t0, cn) in plan:
            blk = xg[:, t0:t0 + cn].reshape(KO, P, cn)
            xd[:, KO * t0:KO * (t0 + cn)] = (
                blk.transpose(1, 0, 2).reshape(P, KO * cn))
        x_g.append(np.ascontiguousarray(xd))

    w1_all = _to_bf16(W1)                       # [E, D, F]
    w2_all = _to_bf16(W2)                       # [E, F, D]
    b1_all = np.asarray(b1, np.float32)         # [E, F]

    in_maps = []
    for c in range(E):
        g, k = 0, c                             # group, F-slice index
        sl = slice(k * FS, (k + 1) * FS)
        w1_c = np.concatenate(
            [_swizzle_rows(np.ascontiguousarray(
                w1_all[e, :, sl])) for e in groups[g]], axis=1)
        w2_c = np.concatenate(
            [_swizzle_rows(np.ascontiguousarray(
                w2_all[e, sl, :])) for e in groups[g]], axis=1)
        b1_c = np.concatenate(
            [np.ascontiguousarray(
                b1_all[e, sl].reshape(FT, P).T) for e in groups[g]], axis=1)
        in_maps.append({
            "x": x_g[g],
            "w1": np.ascontiguousarray(w1_c),
            "w2": np.ascontiguousarray(w2_c),
            "b1": np.ascontiguousarray(b1_c),
        })

    kwargs = {}
    if TRACE:
        import os as _os
        kwargs = dict(trace=True, trace_cores=list(range(E)))
        if _os.environ.get("MOE_TRACE_DIR"):
            _os.makedirs(_os.environ["MOE_TRACE_DIR"], exist_ok=True)
            kwargs["tmpdir"] = _os.environ["MOE_TRACE_DIR"]
    res = bass_utils.run_bass_kernel_spmd(nc, in_maps, list(range(E)), **kwargs)
    global LAST_RESULTS
    LAST_RESULTS = res

    # ---- Combine: per group sum the 4 partial y's, un-swizzle, add b2,
    # apply combine weights, scatter-add
    out = np.zeros((B * S, D), dtype=np.float32)
    b2_f = np.asarray(b2, np.float32)
    for g in range(G):
        acc = np.zeros((P, DT * T), dtype=np.float32)
        for k in range(8):
            acc += res.results[g * 8 + k]["y"].astype(np.float32)
        yg = np.empty((D, T), dtype=np.float32)
        for (j, t0, cn) in plan:
            blk = acc[:, DT * t0:DT * (t0 + cn)].reshape(P, DT, cn)
            yg[:, t0:t0 + cn] = blk.transpose(1, 0, 2).reshape(D, cn)
        for j in range(EPG):
            e = groups[g][j]
            cnt = counts[e]
            y_e = yg[:, off_g[g][j]:off_g[g][j] + cnt].T + b2_f[e]
            out[ids[e]] += cws[e][:, None] * y_e
    return out.reshape(B, S, D)


# revision 19
# speedup vs baseline: 1.0077x; 1.0009x over previous
"""Mixture-of-Experts (B=4, S=2048, D=1024, F=4096, E=8, top-2) on 8 trn2 NeuronCores.

Strategy: full expert F-slicing for perfect load balance.
  Every core processes ALL 16384 dispatched (token, expert) pairs, but only
  a 512-wide slice of the expert hidden dim F (gelu is elementwise, so the
  F-split is exact): partial y = gelu(x @ W1[:, sl] + b1[sl]) @ W2[sl, :].
  The host sums the 8 partial y's, adds b2, applies the gate combine
  weights and scatter-adds. Per-core work is exactly 2048 token-equivalents
  (vs. 2176 = padded max expert count for one-expert-per-core), which puts
  the matmul roofline at 16384*128cyc/2.4GHz = 437us.

  DMA: every transfer is contiguous on BOTH the DRAM and SBUF side (the host
  pre-swizzles x, w1, w2 into per-partition-contiguous blocks; y returns in
  a swizzled layout the host undoes). This gives ~7-16KB DMA runs per
  partition instead of 1KB, which keeps the per-core descriptor rate under
  the fabric limit that throttled a naive row-major version of this split
  (~84MB/core of traffic). Inputs issue on the Sync engine queue, outputs on
  the GpSimd engine queue so outputs never convoy behind input triggers
  whose buffer-free semaphores gate on compute progress.

  PE: chunks (<=512 tokens) are software-pipelined — mm1 of chunk i+1 is
  emitted before mm2 of chunk i so the PE never waits on a chunk's last
  gelu; a few dummy warm-up matmuls on zeroed SBUF run during the initial
  DMA fill so the HAM clock gate is releasing by the time real work starts;
  weight tiles are 2D so LDWEIGHTS keeps Fast-Weight-Load (a 3D stationary
  AP measurably disables it: +30ns per matmul); the final two chunks are
  128 tokens to taper the trailing output drain.

  Measured: 470us (baseline expert-parallel kernel: 495us).
"""

import copy
import sys

import numpy as np

for _p in ("/opt/trn_rl_repo", "/opt/pypackages"):
    if _p not in sys.path:
        sys.path.append(_p)

import ml_dtypes

B, S, D = 4, 2048, 1024
F = 4 * D
E = 8
TOP_K = 2
P = 128
G = 1                # core groups
EPG = E // G         # experts per group (= slots)
FS = F // 8          # 512: per-core F slice (8 cores, one group)
C_CHUNK = 512
N_DUMMY = 10

KO = D // P          # 8 k-subtiles for mm1
FT = FS // P         # 8 f-tiles per expert slice
DT = D // P          # 8 d-tiles of the partial y

# test-harness hooks (left off for grading)
TRACE = False
LAST_RESULTS = None

_compiled = {}


def _split_drain_waits(nc, max_waits=1):
    """This walrus build rejects instructions carrying more than one sync
    wait ("Too many sync wait commands"). Keep one wait on the instruction and
    move the excess onto NoOps inserted right before it on the same engine
    (engines are in-order, so blocking semantics are identical). Updates stay
    on the original instruction — moving them to a trailing NoOp could signal
    before the op's writes land."""
    import concourse.mybir as mybir

    m = nc.m
    new_module = copy.replace(m, functions=[])
    for function in m.functions:
        new_function = copy.replace(function, blocks=[])
        new_function.set_allocations_from_list(function.allocations)
        for block in function.blocks:
            out = []
            for inst in block.instructions:
                si = getattr(inst, "sync_info", None)
                on_wait = list(si.on_wait) if si is not None and si.on_wait else []
                if len(on_wait) > max_waits:
                    engine = getattr(inst, "engine", None)
                    extra, keep = on_wait[max_waits:], on_wait[:max_waits]
                    for j, w in enumerate(extra):
                        out.append(
                            mybir.InstNoOp(
                                name=f"{inst.name}-w{j}",
                                engine=engine,
                                sync_info=mybir.SyncInfo(on_wait=[w], on_update=[]),
                                bass_nofuse=True,
                            )
                        )
                    inst.sync_info = mybir.SyncInfo(
                        on_wait=keep,
                        on_update=list(si.on_update) if si.on_update else [],
                    )
                out.append(inst)
            new_function.blocks.append(copy.replace(block, instructions=out))
        new_module.functions.append(new_function)
    nc.m = new_module
    return nc


def _chunk_plan(caps):
    """Flat list of (slot, t0, cn) chunks. Per-slot chunk sizes balanced
    (<=512, diff<=1); the final chunk overall is carved to 128 tokens to
    shrink the trailing DMA after the last matmul."""
    plan = []
    t0 = 0
    for j, cap in enumerate(caps):
        cap = int(cap)
        if cap == 0:
            continue
        tails = []
        if j == len(caps) - 1 and cap > 640:
            tails = [256]               # taper the final output drain; N>=256
            cap -= 256                  # keeps LDWEIGHTS hidden (N=128 is not)
        n = -(-cap // C_CHUNK)
        base, rem = divmod(cap, n)
        for i in range(n):
            cn = base + (1 if i < rem else 0)
            plan.append((j, t0, cn))
            t0 += cn
        for tail in tails:
            plan.append((j, t0, tail))
            t0 += tail
    return plan


def _build_nc(caps):
    import concourse.bass as bass
    import concourse.mybir as mybir
    from concourse.tile import TileContext

    fp32 = mybir.dt.float32
    bf16 = mybir.dt.bfloat16
    AF = mybir.ActivationFunctionType

    T = int(sum(caps))
    plan = _chunk_plan(caps)

    nc = bass.Bass()
    # All DRAM layouts are host-swizzled so every DMA is contiguous per
    # partition on both sides:
    #   x_d[p, KO*t0 + ko*cn + c]   = x[ko*128+p, t0+c]   (per chunk)
    #   w1_d[p, (j*KO+ko)*FS + f]   = W1slice[j][ko*128+p, f]
    #   w2_d[p, (j*FT+lf)*D + d]    = W2slice[j][lf*128+p, d]
    #   y_d[p, DT*t0 + dt*cn + c]   = y[dt*128+p, t0+c]   (per chunk)
    x_d = nc.declare_dram_parameter("x", [P, KO * T], bf16, isOutput=False)
    w1_d = nc.declare_dram_parameter("w1", [P, EPG * KO * FS], bf16, isOutput=False)
    w2_d = nc.declare_dram_parameter("w2", [P, EPG * FT * D], bf16, isOutput=False)
    b1_d = nc.declare_dram_parameter("b1", [P, EPG * FT], fp32, isOutput=False)
    y_d = nc.declare_dram_parameter("y", [P, DT * T], bf16, isOutput=True)

    with TileContext(nc) as tc:
        with (
            tc.tile_pool(name="wpool", bufs=1) as wpool,
            tc.tile_pool(name="xpool", bufs=3) as xpool,
            tc.tile_pool(name="hpool", bufs=2) as hpool,
            tc.tile_pool(name="ypool", bufs=3) as ypool,
            tc.tile_pool(name="hpsum", bufs=3, space="PSUM") as hpsum,
            tc.tile_pool(name="ypsum", bufs=5, space="PSUM") as ypsum,
        ):
            # --- PE warm-up on zeroed SBUF; result never read. Scratch PSUM
            # is the first "yps" buffer; real groups reuse it behind them.
            dum_w = wpool.tile([P, C_CHUNK], bf16)
            nc.vector.memset(dum_w[:], 0)
            dum_ps = ypsum.tile([P, C_CHUNK], fp32, tag="yps")
            for _ in range(N_DUMMY):
                nc.tensor.matmul(
                    dum_ps[:], dum_w[:, :P], dum_w[:], start=True, stop=True
                )

            w1_t = [None] * EPG    # [P, KO*FS] 2D per slot
            w2_t = [None] * EPG    # [P, FT*D] 2D per slot
            x_sb = [None] * len(plan)

            def load_x(ci, split=1):
                j, t0, cn = plan[ci]
                t = xpool.tile([P, KO * C_CHUNK], bf16, tag="x")
                step = -(-KO // split)
                for k0 in range(0, KO, step):
                    k1 = min(k0 + step, KO)
                    nc.sync.dma_start(
                        t[:, k0 * cn:k1 * cn],
                        x_d[:, KO * t0 + k0 * cn:KO * t0 + k1 * cn])
                x_sb[ci] = t

            def load_w1(j, kos):
                # per-ko pieces: 2KB contiguous runs per partition, and the
                # first matmuls can begin as soon as their piece lands
                if w1_t[j] is None:
                    t1 = wpool.tile([P, KO * FS], bf16, tag=f"w1_{j}")
                    w1_t[j] = t1
                t1 = w1_t[j]
                for ko in kos:
                    nc.sync.dma_start(
                        t1[:, ko * FS:(ko + 1) * FS],
                        w1_d[:, (j * KO + ko) * FS:(j * KO + ko + 1) * FS])

            def load_w2(j):
                t2 = wpool.tile([P, FT * D], bf16, tag=f"w2_{j}")
                for lf in range(FT):
                    nc.sync.dma_start(
                        t2[:, lf * D:(lf + 1) * D],
                        w2_d[:, (j * FT + lf) * D:(j * FT + lf + 1) * D])
                w2_t[j] = t2

            # startup: just enough bytes ahead of each consumer — w1[slot0]
            # ko0 piece and the first half of x0 gate the first real matmul.
            X_AHEAD = 2            # <= xpool bufs - 1
            j0 = plan[0][0]
            load_w1(j0, [0])
            load_x(0, split=4)
            load_w1(j0, range(1, KO))
            b1_sb = wpool.tile([P, EPG * FT], fp32)
            nc.sync.dma_start(b1_sb[:], b1_d[:])
            if len(plan) > 1:
                load_x(1)
            load_w2(j0)
            if len(plan) > 2:
                load_x(2)
            next_w = j0 + 1

            def load_w_through(j):
                nonlocal next_w
                while next_w <= min(j, EPG - 1):
                    load_w1(next_w, range(KO))
                    load_w2(next_w)
                    next_w += 1

            load_w_through(j0 + 1)

            h_of = [None] * len(plan)

            def mm1(ci):
                j, t0, cn = plan[ci]
                h_sb = hpool.tile([P, FT, C_CHUNK], bf16, tag="h")
                for ft in range(FT):
                    h_ps = hpsum.tile([P, C_CHUNK], fp32, tag="hps")
                    for ko in range(KO):
                        nc.tensor.matmul(
                            h_ps[:, :cn],
                            w1_t[j][:, ko * FS + ft * P:ko * FS + (ft + 1) * P],
                            x_sb[ci][:, ko * cn:(ko + 1) * cn],
                            start=(ko == 0),
                            stop=(ko == KO - 1),
                        )
                    nc.scalar.activation(
                        h_sb[:, ft, :cn], h_ps[:, :cn], AF.Gelu,
                        bias=b1_sb[:, j * FT + ft:j * FT + ft + 1],
                    )
                h_of[ci] = h_sb

            def mm2(ci):
                j, t0, cn = plan[ci]
                h_sb = h_of[ci]
                y_sb = ypool.tile([P, DT * C_CHUNK], bf16, tag="y")
                for dt_ in range(DT):
                    y_ps = ypsum.tile([P, C_CHUNK], fp32, tag="yps")
                    for lf in range(FT):
                        nc.tensor.matmul(
                            y_ps[:, :cn],
                            w2_t[j][:, lf * D + dt_ * P:lf * D + (dt_ + 1) * P],
                            h_sb[:, lf, :cn],
                            start=(lf == 0),
                            stop=(lf == FT - 1),
                        )
                    nc.vector.tensor_copy(
                        y_sb[:, dt_ * cn:(dt_ + 1) * cn], y_ps[:, :cn])
                nc.gpsimd.dma_start(
                    y_d[:, DT * t0:DT * (t0 + cn)], y_sb[:, :DT * cn])
                h_of[ci] = None

            mm1(0)
            for ci in range(1, len(plan)):
                if ci + X_AHEAD < len(plan):
                    load_x(ci + X_AHEAD)
                if ci + 1 < len(plan) and plan[ci + 1][0] != plan[ci][0]:
                    load_w_through(plan[ci + 1][0] + 1)
                mm1(ci)
                mm2(ci - 1)
            mm2(len(plan) - 1)

    return _split_drain_waits(nc)


def _to_bf16(a):
    """Fast float32 -> bfloat16 with round-to-nearest-even via bit ops."""
    a = np.ascontiguousarray(a, dtype=np.float32)
    u = a.view(np.uint32)
    r = ((u + 0x7FFF + ((u >> 16) & 1)) >> 16).astype(np.uint16)
    return r.view(ml_dtypes.bfloat16)


def _swizzle_rows(m):
    """[R*128, C] -> [128, R*C] with block r of C cols holding rows
    r*128..r*128+127 (per-partition contiguous layout)."""
    R = m.shape[0] // P
    return np.ascontiguousarray(
        m.reshape(R, P, m.shape[1]).transpose(1, 0, 2).reshape(P, -1))


def kernel(hidden_states, Wg, bg, W1, b1, W2, b2):
    from concourse import bass_utils

    hs = np.ascontiguousarray(hidden_states, dtype=np.float32).reshape(B * S, D)

    # ---- Gate on host (float64): softmax over experts, top-2, renormalize
    logits = hs.astype(np.float64) @ np.asarray(Wg, np.float64).T
    logits += np.asarray(bg, np.float64)
    logits -= logits.max(axis=-1, keepdims=True)
    p = np.exp(logits)
    p /= p.sum(axis=-1, keepdims=True)

    i1 = p.argmax(axis=-1)
    rows = np.arange(B * S)
    p1 = p[rows, i1]
    pm = p.copy()
    pm[rows, i1] = -1.0
    i2 = pm.argmax(axis=-1)
    p2 = p[rows, i2]
    denom = p1 + p2
    g1 = (p1 / denom).astype(np.float32)
    g2 = (p2 / denom).astype(np.float32)

    ids, cws = [], []
    for e in range(E):
        m1 = np.nonzero(i1 == e)[0]
        m2 = np.nonzero(i2 == e)[0]
        ids.append(np.concatenate([m1, m2]))
        cws.append(np.concatenate([g1[m1], g2[m2]]))
    counts = np.array([len(x) for x in ids])

    # ---- Single group: all 8 cores share all experts (F split 8 ways);
    # slot capacities are the exact per-expert counts — perfect balance.
    rank = np.argsort(-counts, kind="stable")
    groups = [[int(rank[j]) for j in range(EPG)]]
    caps = tuple(int(counts[rank[j]]) for j in range(EPG))
    T = int(sum(caps))
    plan = _chunk_plan(caps)

    if caps not in _compiled:
        _compiled[caps] = _build_nc(caps)
    nc = _compiled[caps]

    hs_b = _to_bf16(hs)

    # ---- Per-group token matrices (swizzled per-chunk, shared by 4 cores)
    x_g, off_g = [], []
    for g in range(G):
        xg = np.zeros((D, T), dtype=ml_dtypes.bfloat16)
        offs = []
        t0 = 0
        for j in range(EPG):
            e = groups[g][j]
            cnt = counts[e]
            xg[:, t0:t0 + cnt] = hs_b[ids[e]].T
            offs.append(t0)
            t0 += caps[j]
        off_g.append(offs)
        # swizzle: per chunk ci at t0 with cn cols: [D, cn] -> [128, KO*cn]
        xd = np.empty((P, KO * T), dtype=ml_dtypes.bfloat16)
        for (j, t0, cn) in plan:
            blk = xg[:, t0:t0 + cn].reshape(KO, P, cn)
            xd[:, KO * t0:KO * (t0 + cn)] = (
                blk.transpose(1, 0, 2).reshape(P, KO * cn))
        x_g.append(np.ascontiguousarray(xd))

    w1_all = _to_bf16(W1)                       # [E, D, F]
    w2_all = _to_bf16(W2)                       # [E, F, D]
    b1_all = np.asarray(b1, np.float32)         # [E, F]

    in_maps = []
    for c in range(E):
        g, k = 0, c                             # group, F-slice index
        sl = slice(k * FS, (k + 1) * FS)
        w1_c = np.concatenate(
            [_swizzle_rows(np.ascontiguousarray(
                w1_all[e, :, sl])) for e in groups[g]], axis=1)
        w2_c = np.concatenate(
            [_swizzle_rows(np.ascontiguousarray(
                w2_all[e, sl, :])) for e in groups[g]], axis=1)
        b1_c = np.concatenate(
            [np.ascontiguousarray(
                b1_all[e, sl].reshape(FT, P).T) for e in groups[g]], axis=1)
        in_maps.append({
            "x": x_g[g],
            "w1": np.ascontiguousarray(w1_c),
            "w2": np.ascontiguousarray(w2_c),
            "b1": np.ascontiguousarray(b1_c),
        })

    kwargs = {}
    if TRACE:
        import os as _os
        kwargs = dict(trace=True, trace_cores=list(range(E)))
        if _os.environ.get("MOE_TRACE_DIR"):
            _os.makedirs(_os.environ["MOE_TRACE_DIR"], exist_ok=True)
            kwargs["tmpdir"] = _os.environ["MOE_TRACE_DIR"]
    res = bass_utils.run_bass_kernel_spmd(nc, in_maps, list(range(E)), **kwargs)
    global LAST_RESULTS
    LAST_RESULTS = res

    # ---- Combine: per group sum the 4 partial y's, un-swizzle, add b2,
    # apply combine weights, scatter-add
    out = np.zeros((B * S, D), dtype=np.float32)
    b2_f = np.asarray(b2, np.float32)
    for g in range(G):
        acc = np.zeros((P, DT * T), dtype=np.float32)
        for k in range(8):
            acc += res.results[g * 8 + k]["y"].astype(np.float32)
        yg = np.empty((D, T), dtype=np.float32)
        for (j, t0, cn) in plan:
            blk = acc[:, DT * t0:DT * (t0 + cn)].reshape(P, DT, cn)
            yg[:, t0:t0 + cn] = blk.transpose(1, 0, 2).reshape(D, cn)
        for j in range(EPG):
            e = groups[g][j]
            cnt = counts[e]
            y_e = yg[:, off_g[g][j]:off_g[g][j] + cnt].T + b2_f[e]
            out[ids[e]] += cws[e][:, None] * y_e
    return out.reshape(B, S, D)
